# revision 1
# baseline (speedup 1.0000x reference)
"""CapsNet routing layer (nn_CapsLayer) on 8 Trainium2 NeuronCores.

reference:
    u_hat = einsum("ncoi,bci->bnco", W[0], x)         # B,N,C,O = 1024,2,512,64
    3 dynamic-routing iterations (softmax over n, weighted sum over c,
    squash, agreement update); returns v from iteration 3.

Strategy (in-caps sharded, hardcoded shapes):
  - 8 cores x 64 in-caps, every core sees the full batch. Per-core DMA
    is 64 MiB of x + 8 MiB of W (resident in SBUF) instead of the
    128+ MiB a batch shard would need (W replicated).
  - GEMM is a bf16 hi/lo x3 decomposition (xh*Wh + xh*Wl + xl*Wh, fp32
    PSUM accumulate): ~1e-5 relative accuracy at bf16 PE rate with
    fast-weight-load (128 stationary columns = one 128-sample chunk).
  - The batch is processed as 8 pipelined chunks of 128 samples; each
    chunk's u_hat tile (p=b128, f=(n,c,o), 4 MiB fp32) stays in SBUF for
    all 3 routing iterations. Chunk k+1's GEMM overlaps chunk k's
    routing.
  - The weighted sum over in-caps needs a cross-core reduction: one
    64 KB AllReduce per (chunk, iteration) = 24 total, overlapped with
    other chunks' compute. Every core then computes identical v
    (redundantly) and writes the full output; the host takes core 0's.
  - softmax over N=2 collapses to sigmoid(b0-b1); iteration 1 uses the
    exact coefficients 0.5; iteration 3 skips the dead y/b update.
    Routing is single-instruction TT/TR passes on DVE (GPSIMD offload
    measured net-negative: SBUF port contention stretches DVE slices).
"""
import os
import sys
import types

sys.path.insert(0, "/opt/trn_rl_repo")

import numpy as np
import ml_dtypes
import concourse.bass as bass
import concourse.mybir as mybir
import concourse.tile as tile
from concourse.bass_utils import run_bass_kernel_spmd

BF16NP = ml_dtypes.bfloat16

B, NCAPS, C, ICH, OCH = 1024, 2, 512, 256, 64
ITERATIONS = 3
NCORES = 8
CPC = C // NCORES            # in-caps per core = 64
NBCH = 8                     # batch chunks
BCH = B // NBCH              # samples per chunk = 128
KH = 2                       # K halves (ICH = 2*128)
CG = 8                       # c's per GEMM/DMA group

FP32 = mybir.dt.float32
BF16 = mybir.dt.bfloat16
ADD = mybir.AluOpType.add
MULT = mybir.AluOpType.mult
SUB = mybir.AluOpType.subtract
AF = mybir.ActivationFunctionType
AX = mybir.AxisListType

LAST_EXEC_NS = None


def _install_profile_hook():
    """antenv.axon_hooks is absent in this image; recreate it so
    run_bass_kernel_spmd(trace=True)/BASS_TRACE can report exec_time_ns."""
    if "antenv.axon_hooks" in sys.modules:
        return
    mod = types.ModuleType("antenv.axon_hooks")
    mod._hook = None
    mod.set_axon_ntff_profile_hook = lambda h: setattr(mod, "_hook", h)
    mod.get_axon_ntff_profile_hook = lambda: mod._hook
    sys.modules["antenv.axon_hooks"] = mod
    try:
        from trn_agent_boot.trn_boot import _ntff_profile_via_ctypes

        hook = _ntff_profile_via_ctypes("/opt/axon/libaxon_pjrt.so")
        if hook is not None:
            mod._hook = hook
    except Exception:
        pass


def _split_sync_waits(nc, max_waits=1):
    """walrus setupSyncWait rejects instructions with more than one sem
    wait; hoist extras onto same-engine InstNoOp's placed just before."""
    for f in nc.m.functions:
        for bb in f.blocks:
            out = []
            changed = False
            for inst in bb.instructions:
                si = inst.sync_info
                waits = list(si.on_wait) if si is not None and si.on_wait else []
                if len(waits) > max_waits:
                    extra, keep = waits[:-max_waits], waits[-max_waits:]
                    for g, w in enumerate(extra):
                        out.append(
                            mybir.InstNoOp(
                                name=f"{inst.name}_wsplit{g}",
                                engine=inst.engine,
                                bass_nofuse=True,
                                sync_info=mybir.SyncInfo(on_wait=[w], on_update=[]),
                            )
                        )
                    inst.sync_info = mybir.SyncInfo(
                        on_wait=keep,
                        on_update=list(si.on_update) if si.on_update else [],
                    )
                    changed = True
                out.append(inst)
            if changed:
                bb.instructions = out


def build_kernel(split_waits=True):
    nc = bass.Bass(
        "TRN2", target_bir_lowering=False, debug=False, num_devices=NCORES
    )
    # x shard: [h, i, bchunk, c, b] bf16 (hi and lo)
    xth = nc.dram_tensor("xth", [KH, 128, NBCH, CPC, BCH], BF16, kind="ExternalInput").ap()
    xtl = nc.dram_tensor("xtl", [KH, 128, NBCH, CPC, BCH], BF16, kind="ExternalInput").ap()
    # W shard: [h, i, c, (n,o)] bf16 (hi and lo)
    wth = nc.dram_tensor("wth", [KH, 128, CPC, NCAPS * OCH], BF16, kind="ExternalInput").ap()
    wtl = nc.dram_tensor("wtl", [KH, 128, CPC, NCAPS * OCH], BF16, kind="ExternalInput").ap()
    out = nc.dram_tensor("out", [B, NCAPS, OCH], FP32, kind="ExternalOutput").ap()

    with tile.TileContext(nc) as tc:
        with (
            tc.tile_pool(name="xin", bufs=3) as xpool,
            tc.tile_pool(name="psum", bufs=3, space="PSUM") as pspool,
            tc.tile_pool(name="ubuf", bufs=2) as upool,
            tc.tile_pool(name="dram", bufs=4, space="DRAM") as drpool,
        ):
            # resident W: [h][hi/lo] tiles (128i, c*no)
            wsb = {}
            for h in range(KH):
                for hl, src in (("h", wth), ("l", wtl)):
                    t = nc.alloc_sbuf_tensor(f"w{hl}{h}", [128, CPC * NCAPS * OCH], BF16).ap()
                    nc.sync.dma_start(t[:], src[h].rearrange("i c f -> i (c f)"))
                    wsb[(h, hl)] = t

            # routing tensors, duplicated by chunk parity so even/odd
            # chunks' routing can interleave (hides AllReduce stalls)
            d_all = nc.alloc_sbuf_tensor("d_all", [128, NBCH, CPC], FP32).ap()
            P = 2
            wn = [nc.alloc_sbuf_tensor(f"wscr{p}", [128, CPC, OCH], FP32).ap() for p in range(P)]
            coeff = [nc.alloc_sbuf_tensor(f"coeff{p}", [128, NCAPS, CPC], FP32).ap() for p in range(P)]
            sp = [nc.alloc_sbuf_tensor(f"sp{p}", [128, NCAPS, OCH], FP32).ap() for p in range(P)]
            ss = [nc.alloc_sbuf_tensor(f"ss{p}", [128, NCAPS, OCH], FP32).ap() for p in range(P)]
            s2 = [nc.alloc_sbuf_tensor(f"s2{p}", [128, NCAPS, OCH], FP32).ap() for p in range(P)]
            sq = [nc.alloc_sbuf_tensor(f"sq{p}", [128, NCAPS], FP32).ap() for p in range(P)]
            rr = [nc.alloc_sbuf_tensor(f"rr{p}", [128, NCAPS], FP32).ap() for p in range(P)]
            den = [nc.alloc_sbuf_tensor(f"den{p}", [128, NCAPS], FP32).ap() for p in range(P)]
            rec = [nc.alloc_sbuf_tensor(f"rec{p}", [128, NCAPS], FP32).ap() for p in range(P)]
            mmv = [nc.alloc_sbuf_tensor(f"mmv{p}", [128, NCAPS], FP32).ap() for p in range(P)]
            v = [nc.alloc_sbuf_tensor(f"v{p}", [128, NCAPS, OCH], FP32).ap() for p in range(P)]
            y = [nc.alloc_sbuf_tensor(f"y{p}", [128, NCAPS, CPC], FP32).ap() for p in range(P)]
            dd = [nc.alloc_sbuf_tensor(f"dd{p}", [128, CPC], FP32).ap() for p in range(P)]

            def gemm_chunk(bk):
                u = upool.tile([128, NCAPS, CPC, OCH], FP32, tag="u")
                # ---- GEMM for this chunk ----
                for cg in range(CPC // CG):
                    c0 = cg * CG
                    xt = {}
                    for h in range(KH):
                        for hl, src in (("h", xth), ("l", xtl)):
                            t = xpool.tile([128, CG, BCH], BF16, tag=f"x{hl}{h}")
                            nc.sync.dma_start(t[:], src[h, :, bk, c0 : c0 + CG, :])
                            xt[(h, hl)] = t
                    pg = pspool.tile([BCH, CG, NCAPS * OCH], FP32, tag="pg")
                    for j in range(CG):
                        c = c0 + j
                        terms = []
                        for h in range(KH):
                            wslice_h = wsb[(h, "h")][
                                :, c * NCAPS * OCH : (c + 1) * NCAPS * OCH
                            ]
                            wslice_l = wsb[(h, "l")][
                                :, c * NCAPS * OCH : (c + 1) * NCAPS * OCH
                            ]
                            terms.append((xt[(h, "h")][:, j, :], wslice_h))
                            terms.append((xt[(h, "h")][:, j, :], wslice_l))
                            terms.append((xt[(h, "l")][:, j, :], wslice_h))
                        for ti, (sta, mov) in enumerate(terms):
                            nc.tensor.matmul(
                                pg[:, j, :],
                                lhsT=sta,
                                rhs=mov,
                                start=(ti == 0),
                                stop=(ti == len(terms) - 1),
                            )
                    # PSUM (b, (c,n,o)) -> u (b, (n,c,o)): strided ACT copy
                    nc.scalar.copy(
                        u[:, :, c0 : c0 + CG, :],
                        pg[:].rearrange("b c (n o) -> b n c o", n=NCAPS),
                    )
                return u

            def route_iter(bk, u, it):
                p = bk % 2
                d = d_all[:, bk, :]
                if True:
                    # s_partial = sum_c coeff * u (coeff = 0.5 exactly on it 0)
                    for n in range(NCAPS):
                        if it == 0:
                            nc.vector.tensor_reduce(
                                sp[p][:, n, :].unsqueeze(2),
                                u[:, n].transpose([0, 2, 1]),
                                axis=AX.X,
                                op=ADD,
                            )
                        else:
                            cb = (
                                coeff[p][:, n, :]
                                .unsqueeze(2)
                                .broadcast_to((128, CPC, OCH))
                            )
                            nc.vector.tensor_tensor(wn[p][:], u[:, n], cb, op=MULT)
                            nc.vector.tensor_reduce(
                                sp[p][:, n, :].unsqueeze(2),
                                wn[p][:].transpose([0, 2, 1]),
                                axis=AX.X,
                                op=ADD,
                            )
                    if it == 0:
                        nc.vector.tensor_scalar(sp[p][:], sp[p][:], 0.5, None, op0=MULT)
                    # AllReduce s_partial across the 8 c-shards
                    bi = drpool.tile([128, NCAPS * OCH], FP32, tag="bi")
                    bo = drpool.tile([128, NCAPS * OCH], FP32, tag="bo")
                    nc.sync.dma_start(bi[:], sp[p][:].rearrange("p n o -> p (n o)"))
                    nc.gpsimd.collective_compute(
                        "AllReduce",
                        ADD,
                        replica_groups=[list(range(NCORES))],
                        ins=[bi[:].opt()],
                        outs=[bo[:].opt()],
                    )
                    nc.sync.dma_start(ss[p][:].rearrange("p n o -> p (n o)"), bo[:])

                    # squash: v = ss * sq / ((1+sq) sqrt(sq))
                    nc.vector.tensor_tensor(s2[p][:], ss[p][:], ss[p][:], op=MULT)
                    nc.vector.tensor_reduce(
                        sq[p][:].unsqueeze(2), s2[p][:], axis=AX.X, op=ADD
                    )
                    nc.scalar.activation(rr[p][:], sq[p][:], AF.Sqrt)
                    nc.vector.tensor_scalar(den[p][:], sq[p][:], 1.0, None, op0=ADD)
                    nc.vector.tensor_tensor(den[p][:], den[p][:], rr[p][:], op=MULT)
                    nc.vector.reciprocal(rec[p][:], den[p][:])
                    nc.vector.tensor_tensor(mmv[p][:], sq[p][:], rec[p][:], op=MULT)
                    vb2 = mmv[p][:].unsqueeze(2).broadcast_to((128, NCAPS, OCH))
                    nc.vector.tensor_tensor(v[p][:], ss[p][:], vb2, op=MULT)

                    if it == ITERATIONS - 1:
                        return

                    # y = sum_o u * v
                    for n in range(NCAPS):
                        vb = (
                            v[p][:, n, :]
                            .unsqueeze(1)
                            .broadcast_to((128, CPC, OCH))
                        )
                        nc.vector.tensor_tensor(wn[p][:], u[:, n], vb, op=MULT)
                        nc.vector.tensor_reduce(
                            y[p][:, n, :].unsqueeze(2), wn[p][:], axis=AX.X, op=ADD
                        )
                    # d += y0 - y1 ; coeff = sigmoid(+/-d)
                    nc.vector.tensor_tensor(dd[p][:], y[p][:, 0, :], y[p][:, 1, :], op=SUB)
                    if it == 0:
                        nc.vector.tensor_copy(d, dd[p][:])
                    else:
                        nc.vector.tensor_tensor(d, d, dd[p][:], op=ADD)
                    nc.scalar.activation(coeff[p][:, 0, :], d, AF.Sigmoid)
                    nc.scalar.activation(coeff[p][:, 1, :], d, AF.Sigmoid, scale=-1.0)

            # chunk pairs: interleave iteration emission so one chunk's
            # AllReduce stall hides behind the partner's DVE work
            for pr in range(NBCH // 2):
                bks = (2 * pr, 2 * pr + 1)
                us = {bk: gemm_chunk(bk) for bk in bks}
                for it in range(ITERATIONS):
                    for bk in bks:
                        route_iter(bk, us[bk], it)
                for bk in bks:
                    # every core writes the (identical) chunk result
                    nc.sync.dma_start(
                        out[bk * BCH : (bk + 1) * BCH, :, :], v[bk % 2][:]
                    )

    if split_waits:
        _split_sync_waits(nc)
    return nc


def _prep_inputs(x, W):
    x = np.ascontiguousarray(x, dtype=np.float32)
    W0 = np.ascontiguousarray(W.reshape(NCAPS, C, OCH, ICH), dtype=np.float32)
    xth_cores, xtl_cores, wth_cores, wtl_cores = [], [], [], []
    for k in range(NCORES):
        cs = k * CPC
        xc = x[:, cs : cs + CPC, :]  # (B, 64, 256)
        x6 = xc.reshape(NBCH, BCH, CPC, KH, 128)
        xt = np.ascontiguousarray(x6.transpose(3, 4, 0, 2, 1))  # (h,i,bk,c,b)
        xh = xt.astype(BF16NP)
        xlo = (xt - xh.astype(np.float32)).astype(BF16NP)
        xth_cores.append(xh)
        xtl_cores.append(xlo)
        Wc = W0[:, cs : cs + CPC]  # (2, 64, 64, 256)
        w5 = Wc.reshape(NCAPS, CPC, OCH, KH, 128)
        wt = np.ascontiguousarray(w5.transpose(3, 4, 1, 0, 2)).reshape(
            KH, 128, CPC, NCAPS * OCH
        )
        wh = wt.astype(BF16NP)
        wlo = (wt - wh.astype(np.float32)).astype(BF16NP)
        wth_cores.append(wh)
        wtl_cores.append(wlo)
    return xth_cores, xtl_cores, wth_cores, wtl_cores


_NC_CACHE = {}


def kernel(x, W):
    global LAST_EXEC_NS
    _install_profile_hook()
    if "nc" not in _NC_CACHE:
        _NC_CACHE["nc"] = build_kernel()
    nc = _NC_CACHE["nc"]
    xth, xtl, wth, wtl = _prep_inputs(np.asarray(x), np.asarray(W))
    in_maps = [
        {"xth": xth[k], "xtl": xtl[k], "wth": wth[k], "wtl": wtl[k]}
        for k in range(NCORES)
    ]
    trace = bool(os.environ.get("CAPS_TRACE"))
    res = run_bass_kernel_spmd(nc, in_maps, list(range(NCORES)), trace=trace)
    LAST_EXEC_NS = res.exec_time_ns
    return res.results[0]["out"].astype(np.float32)



# revision 2
# speedup vs baseline: 1.1698x; 1.1698x over previous
"""CapsNet routing layer (nn_CapsLayer) on 8 Trainium2 NeuronCores — v3.

reference:
    u_hat = einsum("ncoi,bci->bnco", W[0], x)         # B,N,C,O = 1024,2,512,64
    3 dynamic-routing iterations (softmax over n, weighted sum over c,
    squash, agreement update); returns v from iteration 3.

v3 = v2 (fp16 single-pass GEMM, packed-fp16 TT routing at DVE 2x,
tree reductions, sigmoid-complement trick, PE-side s_acc, ACT squash)
plus schedule fixes driven by the v2 trace (602us: DVE busy 346us but
idle 223us; comms queue 24 ARs x 16us = 385us saturated):
  - chunk-PAIR AllReduces: 12 collectives of 128KB instead of 24x64KB,
    keeping the comms queue at ~55% of DVE time.
  - software-pipelined pair schedule (2 pairs in flight, iteration
    steps staggered) so every AR has ~30us of other-pair DVE work
    between launch and first consumer.
  - pair 0 computes U = sum_c u with a DVE copy+tree while DVE would
    otherwise idle waiting for the first GEMM; pairs 1-3 keep the PE
    s_acc stream (PE has slack).
  - ACT ops batched by activation-table set (Sigmoid vs Sqrt tables
    conflict; Square/Identity/Copy live in both) and squash smalls run
    at pair level ([128,4] tensors), halving small-op count.
  - W DMA split per c-group so the first matmuls start after ~2us.
"""
import os
import sys
import types

sys.path.insert(0, "/opt/trn_rl_repo")

import numpy as np
import concourse.bass as bass
import concourse.mybir as mybir
import concourse.tile as tile
from concourse.bass_utils import run_bass_kernel_spmd

B, NCAPS, C, ICH, OCH = 1024, 2, 512, 256, 64
ITERATIONS = 3
NCORES = 8
CPC = C // NCORES            # in-caps per core = 64
NBCH = 8                     # batch chunks
BCH = B // NBCH              # samples per chunk = 128
KH = 2                       # K halves (ICH = 2*128)
CG = 8                       # c's per GEMM/DMA group
NO = NCAPS * OCH             # 128
NPAIR = NBCH // 2            # 4 chunk pairs
NSL = 4                      # chunk slots (2 pairs in flight)

FP32 = mybir.dt.float32
FP16 = mybir.dt.float16
ADD = mybir.AluOpType.add
MULT = mybir.AluOpType.mult
SUB = mybir.AluOpType.subtract
AF = mybir.ActivationFunctionType

LAST_EXEC_NS = None


def _install_profile_hook():
    """antenv.axon_hooks is absent in this image; recreate it so
    run_bass_kernel_spmd(trace=True)/BASS_TRACE can report exec_time_ns."""
    if "antenv.axon_hooks" in sys.modules:
        return
    mod = types.ModuleType("antenv.axon_hooks")
    mod._hook = None
    mod.set_axon_ntff_profile_hook = lambda h: setattr(mod, "_hook", h)
    mod.get_axon_ntff_profile_hook = lambda: mod._hook
    sys.modules["antenv.axon_hooks"] = mod
    try:
        from trn_agent_boot.trn_boot import _ntff_profile_via_ctypes

        hook = _ntff_profile_via_ctypes("/opt/axon/libaxon_pjrt.so")
        if hook is not None:
            mod._hook = hook
    except Exception:
        pass


def _split_sync_waits(nc, max_waits=1):
    """walrus setupSyncWait rejects instructions with more than one sem
    wait; hoist extras onto same-engine InstNoOp's placed just before."""
    for f in nc.m.functions:
        for bb in f.blocks:
            out = []
            changed = False
            for inst in bb.instructions:
                si = inst.sync_info
                waits = list(si.on_wait) if si is not None and si.on_wait else []
                if len(waits) > max_waits:
                    extra, keep = waits[:-max_waits], waits[-max_waits:]
                    for g, w in enumerate(extra):
                        out.append(
                            mybir.InstNoOp(
                                name=f"{inst.name}_wsplit{g}",
                                engine=inst.engine,
                                bass_nofuse=True,
                                sync_info=mybir.SyncInfo(on_wait=[w], on_update=[]),
                            )
                        )
                    inst.sync_info = mybir.SyncInfo(
                        on_wait=keep,
                        on_update=list(si.on_update) if si.on_update else [],
                    )
                    changed = True
                out.append(inst)
            if changed:
                bb.instructions = out


def build_kernel(split_waits=True):
    nc = bass.Bass(
        "TRN2", target_bir_lowering=False, debug=False, num_devices=NCORES
    )
    xt = nc.dram_tensor("xt", [KH, 128, NBCH, CPC, BCH], FP16, kind="ExternalInput").ap()
    wt = nc.dram_tensor("wt", [KH, 128, CPC, NO], FP16, kind="ExternalInput").ap()
    out = nc.dram_tensor("out", [B, NCAPS, OCH], FP32, kind="ExternalOutput").ap()

    with tile.TileContext(nc) as tc:
        with (
            tc.tile_pool(name="xin", bufs=3) as xpool,
            tc.tile_pool(name="psum", bufs=2, space="PSUM") as pspool,
            tc.tile_pool(name="sacc", bufs=2, space="PSUM") as sapool,
            tc.tile_pool(name="ubuf", bufs=5) as upool,
            tc.tile_pool(name="dram", bufs=8, space="DRAM") as drpool,
        ):
            # resident W, DMA'd per c-group so first matmuls start early
            wsb = []
            for h in range(KH):
                t = nc.alloc_sbuf_tensor(f"w{h}", [128, CPC * NO], FP16).ap()
                wsb.append(t)
            for cg in range(CPC // CG):
                c0 = cg * CG
                for h in range(KH):
                    nc.sync.dma_start(
                        wsb[h][:, c0 * NO : (c0 + CG) * NO],
                        wt[h, :, c0 : c0 + CG, :].rearrange("i c f -> i (c f)"),
                    )

            # persistent routing state / scratch
            d_all = nc.alloc_sbuf_tensor("d_all", [128, NBCH, CPC], FP16).ap()
            P = 2
            wsc = [nc.alloc_sbuf_tensor(f"wsc{p}", [128, NCAPS, CPC, OCH], FP16).ap() for p in range(P)]
            sgm = [nc.alloc_sbuf_tensor(f"sgm{p}", [128, CPC, OCH], FP16).ap() for p in range(P)]
            dds = [nc.alloc_sbuf_tensor(f"dds{p}", [128, CPC], FP16).ap() for p in range(P)]
            vts = [nc.alloc_sbuf_tensor(f"vt{s}", [128, NCAPS, OCH], FP16).ap() for s in range(NSL)]
            # pair-level tensors (2 pair-slots), z = chunk-in-pair
            QP = 2
            Upr = [nc.alloc_sbuf_tensor(f"Upr{q}", [128, 2, NCAPS, OCH], FP32).ap() for q in range(QP)]
            ssp = [nc.alloc_sbuf_tensor(f"ssp{q}", [128, 2, NCAPS, OCH], FP32).ap() for q in range(QP)]
            pfp = [nc.alloc_sbuf_tensor(f"pfp{q}", [128, 2, NCAPS, OCH], FP32).ap() for q in range(QP)]
            s2p = [nc.alloc_sbuf_tensor(f"s2p{q}", [128, 2, NCAPS, OCH], FP32).ap() for q in range(QP)]
            sab = [nc.alloc_sbuf_tensor(f"sab{q}", [128, 2, NO], FP32).ap() for q in range(QP)]
            sqp = [nc.alloc_sbuf_tensor(f"sqp{q}", [128, 4], FP32).ap() for q in range(QP)]
            rrp = [nc.alloc_sbuf_tensor(f"rrp{q}", [128, 4], FP32).ap() for q in range(QP)]
            dnp = [nc.alloc_sbuf_tensor(f"dnp{q}", [128, 4], FP32).ap() for q in range(QP)]
            rcp = [nc.alloc_sbuf_tensor(f"rcp{q}", [128, 4], FP32).ap() for q in range(QP)]
            mmp = [nc.alloc_sbuf_tensor(f"mmp{q}", [128, 4], FP32).ap() for q in range(QP)]
            m2p = [nc.alloc_sbuf_tensor(f"m2p{q}", [128, 4], FP32).ap() for q in range(QP)]
            vfp = [nc.alloc_sbuf_tensor(f"vfp{q}", [128, 2, NCAPS, OCH], FP32).ap() for q in range(QP)]

            # squash-factor sign/scale consts per (z, n)
            sgn0 = nc.alloc_sbuf_tensor("sgn0", [128, 4], FP32).ap()
            sgn1 = nc.alloc_sbuf_tensor("sgn1", [128, 4], FP32).ap()
            for z in range(2):
                nc.vector.memset(sgn0[:, 2 * z : 2 * z + 1], 0.5)
                nc.vector.memset(sgn0[:, 2 * z + 1 : 2 * z + 2], -0.5)
                nc.vector.memset(sgn1[:, 2 * z : 2 * z + 1], 1.0)
                nc.vector.memset(sgn1[:, 2 * z + 1 : 2 * z + 2], -1.0)

            def allreduce_pair(src, dst):
                bi = drpool.tile([128, 2 * NO], FP32, tag="bi")
                bo = drpool.tile([128, 2 * NO], FP32, tag="bo")
                nc.sync.dma_start(bi[:], src)
                nc.gpsimd.collective_compute(
                    "AllReduce",
                    ADD,
                    replica_groups=[list(range(NCORES))],
                    ins=[bi[:].opt()],
                    outs=[bo[:].opt()],
                )
                nc.sync.dma_start(dst, bo[:])

            def gemm_chunk(bk, with_sacc):
                u = upool.tile([128, NCAPS, CPC, OCH], FP16, tag="u")
                if with_sacc:
                    sacc = sapool.tile([128, NO], FP32, tag="sacc")
                else:
                    sacc = None
                for cg in range(CPC // CG):
                    c0 = cg * CG
                    xtt = []
                    for h in range(KH):
                        t = xpool.tile([128, CG, BCH], FP16, tag=f"x{h}")
                        nc.sync.dma_start(t[:], xt[h, :, bk, c0 : c0 + CG, :])
                        xtt.append(t)
                    pg = pspool.tile([BCH, CG, NO], FP32, tag="pg")
                    for j in range(CG):
                        c = c0 + j
                        for h in range(KH):
                            lhs = xtt[h][:, j, :]
                            rhs = wsb[h][:, c * NO : (c + 1) * NO]
                            nc.tensor.matmul(
                                pg[:, j, :], lhsT=lhs, rhs=rhs,
                                start=(h == 0), stop=(h == KH - 1),
                            )
                            if with_sacc:
                                nc.tensor.matmul(
                                    sacc[:], lhsT=lhs, rhs=rhs,
                                    start=(c == 0 and h == 0),
                                    stop=(c == CPC - 1 and h == KH - 1),
                                    skip_group_check=True,
                                )
                    nc.scalar.copy(
                        u[:, :, c0 : c0 + CG, :],
                        pg[:].rearrange("b c (n o) -> b n c o", n=NCAPS),
                    )
                return u, sacc

            def gemm_pair(q, us, with_sacc=True):
                qp = q % QP
                for z in range(2):
                    bk = 2 * q + z
                    u, sacc = gemm_chunk(bk, with_sacc)
                    us[bk] = u
                    if with_sacc:
                        nc.scalar.copy(sab[qp][:, z, :], sacc[:])
                    else:
                        # DVE copy+tree (startup only, DVE otherwise idle)
                        nc.vector.tensor_copy(wsc[z][:], u[:])
                        tree(wsc[z], CPC, False)
                        nc.vector.tensor_copy(
                            sab[qp][:, z, :].rearrange("p (n o) -> p n o", n=NCAPS),
                            wsc[z][:, :, 0, :],
                        )
                allreduce_pair(
                    sab[qp][:].rearrange("p z f -> p (z f)"),
                    Upr[qp][:].rearrange("p z n o -> p (z n o)"),
                )

            def tree(t, axis_len, o_axis):
                """in-place binary-tree sum over c (o_axis=False) or o."""
                lv = axis_len // 2
                while lv >= 1:
                    if o_axis:
                        a = t[:, :, :, 0:lv]
                        b = t[:, :, :, lv : 2 * lv]
                    else:
                        a = t[:, :, 0:lv, :]
                        b = t[:, :, lv : 2 * lv, :]
                    nc.vector.tensor_tensor(a, a, b, op=ADD)
                    lv //= 2

            def squash_pair(q, s_in, scale, sgn, final=False):
                """pair-level squash: factor=sqrt(sq)/(1+sq); vt=(s*factor)*sgn
                or (final) vfp = s*factor."""
                qp = q % QP
                for z in range(2):
                    for n in range(NCAPS):
                        nc.scalar.activation(
                            s2p[qp][:, z, n, :], s_in[:, z, n, :], AF.Square,
                            scale=float(scale),
                            accum_out=sqp[qp][:, 2 * z + n : 2 * z + n + 1],
                        )
                nc.scalar.activation(rrp[qp][:], sqp[qp][:], AF.Sqrt)
                nc.scalar.activation(dnp[qp][:], sqp[qp][:], AF.Identity, bias=1.0)
                nc.vector.reciprocal(rcp[qp][:], dnp[qp][:])
                nc.vector.tensor_tensor(mmp[qp][:], rrp[qp][:], rcp[qp][:], op=MULT)
                if final:
                    mb = (
                        mmp[qp][:]
                        .rearrange("p (z n) -> p z n", z=2)
                        .unsqueeze(3)
                        .broadcast_to((128, 2, NCAPS, OCH))
                    )
                    nc.vector.tensor_tensor(vfp[qp][:], s_in, mb, op=MULT)
                else:
                    nc.vector.tensor_tensor(m2p[qp][:], mmp[qp][:], sgn[:], op=MULT)
                    for z in range(2):
                        bk = 2 * q + z
                        mb = (
                            m2p[qp][:, 2 * z : 2 * z + 2]
                            .unsqueeze(2)
                            .broadcast_to((128, NCAPS, OCH))
                        )
                        nc.vector.tensor_tensor(vts[bk % NSL][:], s_in[:, z], mb, op=MULT)

            def step_S0(q):
                """iteration-0 squash on AR'd s_acc (s = 0.5*U, folded)."""
                qp = q % QP
                squash_pair(q, Upr[qp][:], 0.5, sgn0)

            def step_A(q, it):
                """y-pass (prev iter), sigma, s-mult/tree, partials, AR."""
                qp = q % QP
                # y-passes (no ACT)
                for z in range(2):
                    bk = 2 * q + z
                    u = us[bk]
                    vb = vts[bk % NSL][:].unsqueeze(2).broadcast_to((128, NCAPS, CPC, OCH))
                    nc.vector.tensor_tensor(wsc[z][:], u[:], vb, op=MULT)
                    tree(wsc[z], OCH, True)
                    d = d_all[:, bk, :].unsqueeze(2)
                    if it == 1:
                        nc.vector.tensor_tensor(
                            d, wsc[z][:, 0, :, 0:1], wsc[z][:, 1, :, 0:1], op=ADD
                        )
                    else:
                        nc.vector.tensor_tensor(
                            dds[z][:].unsqueeze(2), wsc[z][:, 0, :, 0:1],
                            wsc[z][:, 1, :, 0:1], op=ADD,
                        )
                        nc.vector.tensor_tensor(
                            d_all[:, bk, :], d_all[:, bk, :], dds[z][:], op=ADD
                        )
                # sigmas batched (one table set)
                for z in range(2):
                    bk = 2 * q + z
                    db = d_all[:, bk, :].unsqueeze(2).broadcast_to((128, CPC, OCH))
                    nc.scalar.activation(sgm[z][:], db, AF.Sigmoid)
                # s-mult + s-tree + extract partials
                for z in range(2):
                    sb = sgm[z][:].unsqueeze(1).broadcast_to((128, NCAPS, CPC, OCH))
                    nc.vector.tensor_tensor(wsc[z][:], us[2 * q + z][:], sb, op=MULT)
                    tree(wsc[z], CPC, False)
                    nc.vector.tensor_copy(pfp[qp][:, z], wsc[z][:, :, 0, :])
                allreduce_pair(
                    pfp[qp][:].rearrange("p z n o -> p (z n o)"),
                    ssp[qp][:].rearrange("p z n o -> p (z n o)"),
                )

            def step_B(q, it):
                """post-AR: sigmoid-complement fix, squash (or final+store)."""
                qp = q % QP
                nc.vector.tensor_tensor(
                    ssp[qp][:, :, 1, :], Upr[qp][:, :, 1, :], ssp[qp][:, :, 1, :],
                    op=SUB,
                )
                if it < ITERATIONS - 1:
                    squash_pair(q, ssp[qp][:], 1.0, sgn1)
                else:
                    squash_pair(q, ssp[qp][:], 1.0, None, final=True)
                    for z in range(2):
                        bk = 2 * q + z
                        nc.sync.dma_start(
                            out[bk * BCH : (bk + 1) * BCH, :, :], vfp[qp][:, z]
                        )

            # ---- software-pipelined pair schedule (2 pairs in flight) ----
            # pair q+2's gemm (whose AR0 rewrites Upr/sab slot q%2) must be
            # emitted after step_B(q,2), pair q's last read of that slot.
            us = {}
            gemm_pair(0, us, with_sacc=False)
            gemm_pair(1, us)
            step_S0(0); step_A(0, 1)
            step_S0(1); step_A(1, 1)
            step_B(0, 1); step_A(0, 2)
            step_B(1, 1); step_A(1, 2)
            step_B(0, 2); gemm_pair(2, us)
            step_B(1, 2); gemm_pair(3, us)
            step_S0(2); step_A(2, 1)
            step_S0(3); step_A(3, 1)
            step_B(2, 1); step_A(2, 2)
            step_B(3, 1); step_A(3, 2)
            step_B(2, 2)
            step_B(3, 2)

    if split_waits:
        _split_sync_waits(nc)
    return nc


def _prep_inputs(x, W):
    x = np.ascontiguousarray(x, dtype=np.float32)
    W0 = np.ascontiguousarray(W.reshape(NCAPS, C, OCH, ICH), dtype=np.float32)
    xt_cores, wt_cores = [], []
    for k in range(NCORES):
        cs = k * CPC
        xc = x[:, cs : cs + CPC, :]  # (B, 64, 256)
        x6 = xc.reshape(NBCH, BCH, CPC, KH, 128)
        xtc = np.ascontiguousarray(x6.transpose(3, 4, 0, 2, 1)).astype(np.float16)
        xt_cores.append(xtc)
        Wc = W0[:, cs : cs + CPC]  # (2, 64, 64, 256) [n,c,o,i]
        w5 = Wc.reshape(NCAPS, CPC, OCH, KH, 128)
        wtc = np.ascontiguousarray(w5.transpose(3, 4, 1, 0, 2)).reshape(
            KH, 128, CPC, NO
        ).astype(np.float16)
        wt_cores.append(wtc)
    return xt_cores, wt_cores


_NC_CACHE = {}


def kernel(x, W):
    global LAST_EXEC_NS
    _install_profile_hook()
    if "nc" not in _NC_CACHE:
        _NC_CACHE["nc"] = build_kernel()
    nc = _NC_CACHE["nc"]
    xtc, wtc = _prep_inputs(np.asarray(x), np.asarray(W))
    in_maps = [{"xt": xtc[k], "wt": wtc[k]} for k in range(NCORES)]
    trace = bool(os.environ.get("CAPS_TRACE"))
    res = run_bass_kernel_spmd(nc, in_maps, list(range(NCORES)), trace=trace)
    LAST_EXEC_NS = res.exec_time_ns
    return res.results[0]["out"].astype(np.float32)
